# revision 11
# baseline (speedup 1.0000x reference)
"""Trainium2 Bass kernel for nn_ConcatLayer_57982058496361 (topk_masking).

Per row of 9 floats (3 groups g of 3 elements [a,b,c]):
  M_g  = max(a,b,c)
  E0_g = (a == M_g), E2_g = (c == M_g)        strict-argmax flags (ties are
  mi_g = E0_g - E2_g                          measure-zero in this data)
  s3   = mi_0 + mi_1 + mi_2
  sc   = sign(s3) * |mi_1|                    in {-1,0,1}
  kp_g = (mi_g == sc)
  vals_g = kp_g * M_g      (for kept groups the reference's x_g[1-sc]
                            always equals the group max M_g)
  wm2  = max_g vals_g
  m_g  = (vals_g == wm2) & (vals_g != 0)
  out  = x_w for the winning group (g=0 priority on ties), else zeros

GPSIMD shares its SBUF port with the Vector engine (concurrent use just
splits the same bandwidth at a worse rate), so all tensor-tensor work
stays on DVE; the mask algebra runs in dense bf16 to hit the DVE 2x
mode, and ACT (separate port) takes the unary sign/square ops.

Data-parallel over 8 NeuronCores; each core processes N/8 rows.
"""

import os
import numpy as np

N_ROWS = 8388608
N_CORES = 8
ROWS_PER_CORE = N_ROWS // N_CORES  # 1048576
P = 128
F = 512                      # rows per partition per tile
TILE_ROWS = P * F
TILES = ROWS_PER_CORE // TILE_ROWS

GP_OMULT = bool(int(os.environ.get("GP_OMULT", "0")))

LAST_EXEC_NS = None
LAST_RESULTS = None
_CACHE = {}


def _register_eqnz():
    """Fused DVE op: out = (in0 == in1) & (in0 != 0)."""
    import concourse.dve_ops as dops
    from concourse.dve_spec import Spec, Src0, Src1, Zero, eq, ne, lower
    from concourse.dve_uop import DveOpSpec

    for o in dops.OPS:
        if o.name == "EQNZ_ANT":
            return o
    spec = Spec(
        body=eq(Src0, Src1) & ne(Src0, Zero),
        reference=lambda in0, in1: ((in0 == in1) & (in0 != 0)).astype(np.float32),
    )
    opcode = dops._CUSTOM_DVE_ROW_BASE + len(dops.OPS)
    shas = {
        v: DveOpSpec(
            name="EQNZ_ANT", opcode=opcode, uops=lower(spec, ver=v), rd1_en=True
        ).sha(v)
        for v in ("v3", "v4")
    }
    op = dops.DveOp("EQNZ_ANT", spec, subdim=False, uops_sha=shas)
    dops.OPS.append(op)
    dops._SUB_OPCODE_FOR_NAME[op.name] = opcode
    dops.CUSTOM_DVE_SPECS[op.name] = spec
    return op


def _build_nc():
    import concourse.bacc as bacc
    import concourse.mybir as mybir
    from concourse.tile import TileContext

    f32 = mybir.dt.float32
    bf16 = mybir.dt.bfloat16
    u8 = mybir.dt.uint8
    Alu = mybir.AluOpType
    EQNZ = _register_eqnz()

    nc = bacc.Bacc(
        "TRN2",
        target_bir_lowering=False,
        debug=False,
        num_devices=N_CORES,
    )
    x_d = nc.dram_tensor("inputs", [ROWS_PER_CORE, 9], f32, kind="ExternalInput")
    o_d = nc.dram_tensor("out", [ROWS_PER_CORE, 3], f32, kind="ExternalOutput")
    xt = x_d.rearrange("(t p f) e -> t p f e", p=P, f=F)  # [T,128,F,9]
    ot = o_d.rearrange("(t p f) e -> t p f e", p=P, f=F)  # [T,128,F,3]

    with TileContext(nc) as tc:
        with tc.tile_pool(name="iox", bufs=3) as iox, \
             tc.tile_pool(name="ioo", bufs=2) as ioo, \
             tc.tile_pool(name="tmp", bufs=3) as tp:
            for t in range(TILES):
                x = iox.tile([P, F, 9], f32, tag="x")
                nc.sync.dma_start(x[:], xt[t])
                x4 = x[:].rearrange("p f (g e) -> p f g e", g=3)
                a_s = x4[:, :, :, 0]   # [P,F,3] strided
                b_s = x4[:, :, :, 1]
                c_s = x4[:, :, :, 2]

                # --- group max and argmax flags (DVE, f32 -> bf16 flags) ---
                q = tp.tile([P, F, 3], f32, tag="q")
                nc.vector.tensor_tensor(q[:], b_s, c_s, Alu.max)
                M = tp.tile([P, F, 3], f32, tag="M")
                nc.vector.tensor_tensor(M[:], a_s, q[:], Alu.max)
                E0 = tp.tile([P, F, 3], bf16, tag="E0")
                nc.vector.tensor_tensor(E0[:], a_s, M[:], Alu.is_equal)
                E2 = tp.tile([P, F, 3], bf16, tag="E2")
                nc.vector.tensor_tensor(E2[:], c_s, M[:], Alu.is_equal)

                # --- mask algebra (bf16; dense ops hit DVE 2x mode) ---
                mi = tp.tile([P, F, 3], bf16, tag="mi")
                nc.vector.tensor_tensor(mi[:], E0[:], E2[:], Alu.subtract)
                s3a = tp.tile([P, F], bf16, tag="s3a")
                nc.vector.tensor_tensor(s3a[:], mi[:, :, 0], mi[:, :, 1], Alu.add)
                s3 = tp.tile([P, F], bf16, tag="s3")
                nc.vector.tensor_tensor(s3[:], s3a[:], mi[:, :, 2], Alu.add)
                sg = tp.tile([P, F], bf16, tag="sg")
                nc.scalar.sign(sg[:], s3[:])                       # ACT
                am = tp.tile([P, F], bf16, tag="am")
                nc.scalar.square(am[:], mi[:, :, 1])               # ACT
                sc = tp.tile([P, F], bf16, tag="sc")
                nc.vector.tensor_tensor(sc[:], sg[:], am[:], Alu.mult)
                scb = tp.tile([P, F, 3], bf16, tag="scb")
                nc.scalar.copy(scb[:], sc[:].broadcast_to((P, F, 3)))  # ACT
                kp = tp.tile([P, F, 3], bf16, tag="kp")
                nc.vector.tensor_tensor(kp[:], mi[:], scb[:], Alu.is_equal)

                # --- vals and tournament (f32) ---
                vals = tp.tile([P, F, 3], f32, tag="vals")
                nc.vector.tensor_tensor(vals[:], kp[:], M[:], Alu.mult)
                v01 = tp.tile([P, F], f32, tag="v01")
                nc.vector.tensor_tensor(v01[:], vals[:, :, 0], vals[:, :, 1], Alu.max)
                wm2 = tp.tile([P, F], f32, tag="wm2")
                nc.vector.tensor_tensor(wm2[:], v01[:], vals[:, :, 2], Alu.max)
                m = tp.tile([P, F, 3], u8, tag="m")
                nc.vector._custom_dve(
                    EQNZ, out=m[:], in0=vals[:], in1=wm2[:].broadcast_to((P, F, 3))
                )

                # --- output: winning group's 3-vector (g0 priority last) ---
                o = ioo.tile([P, F, 3], f32, tag="o")
                eng = nc.gpsimd if GP_OMULT else nc.vector
                eng.tensor_tensor(
                    o[:], m[:, :, 2].broadcast_to((P, F, 3)), x4[:, :, 2, :], Alu.mult
                )
                nc.vector.copy_predicated(
                    o[:], m[:, :, 1].broadcast_to((P, F, 3)), x4[:, :, 1, :]
                )
                nc.vector.copy_predicated(
                    o[:], m[:, :, 0].broadcast_to((P, F, 3)), x4[:, :, 0, :]
                )
                nc.sync.dma_start(ot[t], o[:])
    nc.compile()
    return nc


def _run(full_inputs: np.ndarray, trace: bool = False):
    global LAST_EXEC_NS, LAST_RESULTS
    from concourse.bass_utils import run_bass_kernel_spmd

    if "nc" not in _CACHE:
        _CACHE["nc"] = _build_nc()
    nc = _CACHE["nc"]

    shards = full_inputs.reshape(N_CORES, ROWS_PER_CORE, 9)
    in_maps = [{"inputs": np.ascontiguousarray(shards[i])} for i in range(N_CORES)]
    res = run_bass_kernel_spmd(nc, in_maps, list(range(N_CORES)), trace=trace)
    LAST_EXEC_NS = res.exec_time_ns
    LAST_RESULTS = res
    out = np.concatenate([res.results[i]["out"] for i in range(N_CORES)], axis=0)
    return out


def kernel(inputs: np.ndarray) -> np.ndarray:
    inputs = np.ascontiguousarray(np.asarray(inputs, dtype=np.float32))
    assert inputs.shape == (N_ROWS, 9), inputs.shape
    trace = bool(int(os.environ.get("BASS_KERNEL_TRACE", "0")))
    return _run(inputs, trace=trace)


# revision 12
# speedup vs baseline: 1.0543x; 1.0543x over previous
"""Trainium2 Bass kernel for nn_ConcatLayer_57982058496361 (topk_masking).

Per row of 9 floats (3 groups g of 3 elements [a,b,c]):
  M_g  = max(a,b,c)
  E0_g = (a == M_g), E2_g = (c == M_g)        strict-argmax flags (ties are
  mi_g = E0_g - E2_g                          measure-zero in this data)
  s3   = mi_0 + mi_1 + mi_2
  sc   = sign(s3) * |mi_1|                    in {-1,0,1}
  kp_g = (mi_g == sc)
  vals_g = kp_g * M_g      (for kept groups the reference's x_g[1-sc]
                            always equals the group max M_g)
  wm2  = max_g vals_g
  m_g  = (vals_g == wm2) & (vals_g != 0)
  out  = x_w for the winning group (g=0 priority on ties), else zeros

GPSIMD shares its SBUF port with the Vector engine (concurrent use just
splits the same bandwidth at a worse rate), so all tensor-tensor work
stays on DVE; the mask algebra runs in dense bf16 to hit the DVE 2x
mode, and ACT (separate port) takes the unary sign/square ops.

Data-parallel over 8 NeuronCores; each core processes N/8 rows.
"""

import os
import numpy as np

N_ROWS = 8388608
N_CORES = 8
ROWS_PER_CORE = N_ROWS // N_CORES  # 1048576
P = 128
F = 512                      # rows per partition per tile
TILE_ROWS = P * F
TILES = ROWS_PER_CORE // TILE_ROWS

GP_OMULT = bool(int(os.environ.get("GP_OMULT", "0")))

LAST_EXEC_NS = None
LAST_RESULTS = None
_CACHE = {}


def _register_eqnz():
    """Fused DVE op: out = (in0 == in1) & (in0 != 0)."""
    import concourse.dve_ops as dops
    from concourse.dve_spec import Spec, Src0, Src1, Zero, eq, ne, lower
    from concourse.dve_uop import DveOpSpec

    for o in dops.OPS:
        if o.name == "EQNZ_ANT":
            return o
    spec = Spec(
        body=eq(Src0, Src1) & ne(Src0, Zero),
        reference=lambda in0, in1: ((in0 == in1) & (in0 != 0)).astype(np.float32),
    )
    opcode = dops._CUSTOM_DVE_ROW_BASE + len(dops.OPS)
    shas = {
        v: DveOpSpec(
            name="EQNZ_ANT", opcode=opcode, uops=lower(spec, ver=v), rd1_en=True
        ).sha(v)
        for v in ("v3", "v4")
    }
    op = dops.DveOp("EQNZ_ANT", spec, subdim=False, uops_sha=shas)
    dops.OPS.append(op)
    dops._SUB_OPCODE_FOR_NAME[op.name] = opcode
    dops.CUSTOM_DVE_SPECS[op.name] = spec
    return op


def _build_nc():
    import concourse.bacc as bacc
    import concourse.mybir as mybir
    from concourse.tile import TileContext

    f32 = mybir.dt.float32
    bf16 = mybir.dt.bfloat16
    u8 = mybir.dt.uint8
    Alu = mybir.AluOpType
    EQNZ = _register_eqnz()

    nc = bacc.Bacc(
        "TRN2",
        target_bir_lowering=False,
        debug=False,
        num_devices=N_CORES,
    )
    x_d = nc.dram_tensor("inputs", [ROWS_PER_CORE, 9], f32, kind="ExternalInput")
    o_d = nc.dram_tensor("out", [ROWS_PER_CORE, 3], f32, kind="ExternalOutput")
    xt = x_d.rearrange("(t p f) e -> t p f e", p=P, f=F)  # [T,128,F,9]
    ot = o_d.rearrange("(t p f) e -> t p f e", p=P, f=F)  # [T,128,F,3]

    with TileContext(nc) as tc:
        with tc.tile_pool(name="iox", bufs=4) as iox, \
             tc.tile_pool(name="ioo", bufs=2) as ioo, \
             tc.tile_pool(name="tmp", bufs=2) as tp:
            for t in range(TILES):
                x = iox.tile([P, F, 9], f32, tag="x")
                nc.sync.dma_start(x[:], xt[t])
                x4 = x[:].rearrange("p f (g e) -> p f g e", g=3)
                a_s = x4[:, :, :, 0]   # [P,F,3] strided
                b_s = x4[:, :, :, 1]
                c_s = x4[:, :, :, 2]

                # --- group max and argmax flags (DVE, f32 -> bf16 flags) ---
                q = tp.tile([P, F, 3], f32, tag="q")
                nc.vector.tensor_tensor(q[:], b_s, c_s, Alu.max)
                M = tp.tile([P, F, 3], f32, tag="M")
                nc.vector.tensor_tensor(M[:], a_s, q[:], Alu.max)
                E0 = tp.tile([P, F, 3], bf16, tag="E0")
                nc.vector.tensor_tensor(E0[:], a_s, M[:], Alu.is_equal)
                E2 = tp.tile([P, F, 3], bf16, tag="E2")
                nc.vector.tensor_tensor(E2[:], c_s, M[:], Alu.is_equal)

                # --- mask algebra (bf16; dense ops hit DVE 2x mode) ---
                mi = tp.tile([P, F, 3], bf16, tag="mi")
                nc.vector.tensor_tensor(mi[:], E0[:], E2[:], Alu.subtract)
                s3a = tp.tile([P, F], bf16, tag="s3a")
                nc.vector.tensor_tensor(s3a[:], mi[:, :, 0], mi[:, :, 1], Alu.add)
                s3 = tp.tile([P, F], bf16, tag="s3")
                nc.vector.tensor_tensor(s3[:], s3a[:], mi[:, :, 2], Alu.add)
                sg = tp.tile([P, F], bf16, tag="sg")
                nc.scalar.sign(sg[:], s3[:])                       # ACT
                am = tp.tile([P, F], bf16, tag="am")
                nc.scalar.square(am[:], mi[:, :, 1])               # ACT
                sc = tp.tile([P, F], bf16, tag="sc")
                nc.vector.tensor_tensor(sc[:], sg[:], am[:], Alu.mult)
                scb = tp.tile([P, F, 3], bf16, tag="scb")
                nc.scalar.copy(scb[:], sc[:].broadcast_to((P, F, 3)))  # ACT
                kp = tp.tile([P, F, 3], bf16, tag="kp")
                nc.vector.tensor_tensor(kp[:], mi[:], scb[:], Alu.is_equal)

                # --- vals and tournament (f32) ---
                vals = tp.tile([P, F, 3], f32, tag="vals")
                nc.vector.tensor_tensor(vals[:], kp[:], M[:], Alu.mult)
                v01 = tp.tile([P, F], f32, tag="v01")
                nc.vector.tensor_tensor(v01[:], vals[:, :, 0], vals[:, :, 1], Alu.max)
                wm2 = tp.tile([P, F], f32, tag="wm2")
                nc.vector.tensor_tensor(wm2[:], v01[:], vals[:, :, 2], Alu.max)
                m = tp.tile([P, F, 3], u8, tag="m")
                nc.vector._custom_dve(
                    EQNZ, out=m[:], in0=vals[:], in1=wm2[:].broadcast_to((P, F, 3))
                )

                # --- output: winning group's 3-vector (g0 priority last) ---
                o = ioo.tile([P, F, 3], f32, tag="o")
                eng = nc.gpsimd if GP_OMULT else nc.vector
                eng.tensor_tensor(
                    o[:], m[:, :, 2].broadcast_to((P, F, 3)), x4[:, :, 2, :], Alu.mult
                )
                nc.vector.copy_predicated(
                    o[:], m[:, :, 1].broadcast_to((P, F, 3)), x4[:, :, 1, :]
                )
                nc.vector.copy_predicated(
                    o[:], m[:, :, 0].broadcast_to((P, F, 3)), x4[:, :, 0, :]
                )
                nc.sync.dma_start(ot[t], o[:])
    nc.compile()
    return nc


def _run(full_inputs: np.ndarray, trace: bool = False):
    global LAST_EXEC_NS, LAST_RESULTS
    from concourse.bass_utils import run_bass_kernel_spmd

    if "nc" not in _CACHE:
        _CACHE["nc"] = _build_nc()
    nc = _CACHE["nc"]

    shards = full_inputs.reshape(N_CORES, ROWS_PER_CORE, 9)
    in_maps = [{"inputs": np.ascontiguousarray(shards[i])} for i in range(N_CORES)]
    res = run_bass_kernel_spmd(nc, in_maps, list(range(N_CORES)), trace=trace)
    LAST_EXEC_NS = res.exec_time_ns
    LAST_RESULTS = res
    out = np.concatenate([res.results[i]["out"] for i in range(N_CORES)], axis=0)
    return out


def kernel(inputs: np.ndarray) -> np.ndarray:
    inputs = np.ascontiguousarray(np.asarray(inputs, dtype=np.float32))
    assert inputs.shape == (N_ROWS, 9), inputs.shape
    trace = bool(int(os.environ.get("BASS_KERNEL_TRACE", "0")))
    return _run(inputs, trace=trace)


# revision 13
# speedup vs baseline: 1.0622x; 1.0075x over previous
"""Trainium2 Bass kernel for nn_ConcatLayer_57982058496361 (topk_masking).

Per row of 9 floats (3 groups g of 3 elements [a,b,c]):
  M_g  = max(a,b,c)
  E0_g = (a == M_g), E2_g = (c == M_g)        strict-argmax flags (ties are
  mi_g = E0_g - E2_g                          measure-zero in this data)
  s3   = mi_0 + mi_1 + mi_2
  sc   = sign(s3) * |mi_1|                    in {-1,0,1}
  kp_g = (mi_g == sc)
  vals_g = kp_g * M_g      (for kept groups the reference's x_g[1-sc]
                            always equals the group max M_g)
  wm2  = max_g vals_g
  m_g  = (vals_g == wm2) & (vals_g != 0)
  out  = x_w for the winning group (g=0 priority on ties), else zeros

GPSIMD shares its SBUF port with the Vector engine (concurrent use just
splits the same bandwidth at a worse rate), so all tensor-tensor work
stays on DVE; the mask algebra runs in dense bf16 to hit the DVE 2x
mode, and ACT (separate port) takes the unary sign/square ops.

Data-parallel over 8 NeuronCores; each core processes N/8 rows.
"""

import os
import numpy as np

N_ROWS = 8388608
N_CORES = 8
ROWS_PER_CORE = N_ROWS // N_CORES  # 1048576
P = 128
F = 512                      # rows per partition per tile
TILE_ROWS = P * F
TILES = ROWS_PER_CORE // TILE_ROWS

GP_OMULT = bool(int(os.environ.get("GP_OMULT", "0")))

LAST_EXEC_NS = None
LAST_RESULTS = None
_CACHE = {}


def _register_eqnz():
    """Fused DVE op: out = (in0 == in1) & (in0 != 0)."""
    import concourse.dve_ops as dops
    from concourse.dve_spec import Spec, Src0, Src1, Zero, eq, ne, lower
    from concourse.dve_uop import DveOpSpec

    for o in dops.OPS:
        if o.name == "EQNZ_ANT":
            return o
    spec = Spec(
        body=eq(Src0, Src1) & ne(Src0, Zero),
        reference=lambda in0, in1: ((in0 == in1) & (in0 != 0)).astype(np.float32),
    )
    opcode = dops._CUSTOM_DVE_ROW_BASE + len(dops.OPS)
    shas = {
        v: DveOpSpec(
            name="EQNZ_ANT", opcode=opcode, uops=lower(spec, ver=v), rd1_en=True
        ).sha(v)
        for v in ("v3", "v4")
    }
    op = dops.DveOp("EQNZ_ANT", spec, subdim=False, uops_sha=shas)
    dops.OPS.append(op)
    dops._SUB_OPCODE_FOR_NAME[op.name] = opcode
    dops.CUSTOM_DVE_SPECS[op.name] = spec
    return op


def _build_nc():
    import concourse.bacc as bacc
    import concourse.mybir as mybir
    from concourse.tile import TileContext

    f32 = mybir.dt.float32
    bf16 = mybir.dt.bfloat16
    u8 = mybir.dt.uint8
    Alu = mybir.AluOpType
    EQNZ = _register_eqnz()

    nc = bacc.Bacc(
        "TRN2",
        target_bir_lowering=False,
        debug=False,
        num_devices=N_CORES,
    )
    x_d = nc.dram_tensor("inputs", [ROWS_PER_CORE, 9], f32, kind="ExternalInput")
    o_d = nc.dram_tensor("out", [ROWS_PER_CORE, 3], f32, kind="ExternalOutput")
    xt = x_d.rearrange("(t p f) e -> t p f e", p=P, f=F)  # [T,128,F,9]
    ot = o_d.rearrange("(t p f) e -> t p f e", p=P, f=F)  # [T,128,F,3]

    with TileContext(nc) as tc:
        with tc.tile_pool(name="iox", bufs=3) as iox, \
             tc.tile_pool(name="ioo", bufs=2) as ioo, \
             tc.tile_pool(name="tmp", bufs=3) as tp:
            for t in range(TILES):
                x = iox.tile([P, F, 9], f32, tag="x")
                nc.sync.dma_start(x[:], xt[t])
                x4 = x[:].rearrange("p f (g e) -> p f g e", g=3)
                a_s = x4[:, :, :, 0]   # [P,F,3] strided
                b_s = x4[:, :, :, 1]
                c_s = x4[:, :, :, 2]

                # --- group max and argmax flags (DVE, f32 -> bf16 flags) ---
                q = tp.tile([P, F, 3], f32, tag="q")
                nc.vector.tensor_tensor(q[:], b_s, c_s, Alu.max)
                M = tp.tile([P, F, 3], f32, tag="M")
                nc.vector.tensor_tensor(M[:], a_s, q[:], Alu.max)
                E0 = tp.tile([P, F, 3], bf16, tag="E0")
                nc.vector.tensor_tensor(E0[:], a_s, M[:], Alu.is_equal)
                E2 = tp.tile([P, F, 3], bf16, tag="E2")
                nc.vector.tensor_tensor(E2[:], c_s, M[:], Alu.is_equal)

                # --- mask algebra (bf16; dense ops hit DVE 2x mode) ---
                mi = tp.tile([P, F, 3], bf16, tag="mi")
                nc.vector.tensor_tensor(mi[:], E0[:], E2[:], Alu.subtract)
                s3a = tp.tile([P, F], bf16, tag="s3a")
                nc.vector.tensor_tensor(s3a[:], mi[:, :, 0], mi[:, :, 1], Alu.add)
                s3 = tp.tile([P, F], bf16, tag="s3")
                nc.vector.tensor_tensor(s3[:], s3a[:], mi[:, :, 2], Alu.add)
                sg = tp.tile([P, F], bf16, tag="sg")
                nc.scalar.sign(sg[:], s3[:])                       # ACT
                am = tp.tile([P, F], bf16, tag="am")
                nc.scalar.square(am[:], mi[:, :, 1])               # ACT
                sc = tp.tile([P, F], bf16, tag="sc")
                nc.vector.tensor_tensor(sc[:], sg[:], am[:], Alu.mult)
                scb = tp.tile([P, F, 3], bf16, tag="scb")
                nc.scalar.copy(scb[:], sc[:].broadcast_to((P, F, 3)))  # ACT
                kp = tp.tile([P, F, 3], bf16, tag="kp")
                nc.vector.tensor_tensor(kp[:], mi[:], scb[:], Alu.is_equal)

                # --- vals and tournament (f32) ---
                vals = tp.tile([P, F, 3], f32, tag="vals")
                nc.vector.tensor_tensor(vals[:], kp[:], M[:], Alu.mult)
                v01 = tp.tile([P, F], f32, tag="v01")
                nc.vector.tensor_tensor(v01[:], vals[:, :, 0], vals[:, :, 1], Alu.max)
                wm2 = tp.tile([P, F], f32, tag="wm2")
                nc.vector.tensor_tensor(wm2[:], v01[:], vals[:, :, 2], Alu.max)
                m = tp.tile([P, F, 3], u8, tag="m")
                nc.vector._custom_dve(
                    EQNZ, out=m[:], in0=vals[:], in1=wm2[:].broadcast_to((P, F, 3))
                )

                # --- output: winning group's 3-vector (g0 priority last) ---
                o = ioo.tile([P, F, 3], f32, tag="o")
                eng = nc.gpsimd if GP_OMULT else nc.vector
                eng.tensor_tensor(
                    o[:], m[:, :, 2].broadcast_to((P, F, 3)), x4[:, :, 2, :], Alu.mult
                )
                nc.vector.copy_predicated(
                    o[:], m[:, :, 1].broadcast_to((P, F, 3)), x4[:, :, 1, :]
                )
                nc.vector.copy_predicated(
                    o[:], m[:, :, 0].broadcast_to((P, F, 3)), x4[:, :, 0, :]
                )
                nc.sync.dma_start(ot[t], o[:])
    nc.compile()
    return nc


def _run(full_inputs: np.ndarray, trace: bool = False):
    global LAST_EXEC_NS, LAST_RESULTS
    from concourse.bass_utils import run_bass_kernel_spmd

    if "nc" not in _CACHE:
        _CACHE["nc"] = _build_nc()
    nc = _CACHE["nc"]

    shards = full_inputs.reshape(N_CORES, ROWS_PER_CORE, 9)
    in_maps = [{"inputs": np.ascontiguousarray(shards[i])} for i in range(N_CORES)]
    res = run_bass_kernel_spmd(nc, in_maps, list(range(N_CORES)), trace=trace)
    LAST_EXEC_NS = res.exec_time_ns
    LAST_RESULTS = res
    out = np.concatenate([res.results[i]["out"] for i in range(N_CORES)], axis=0)
    return out


def kernel(inputs: np.ndarray) -> np.ndarray:
    inputs = np.ascontiguousarray(np.asarray(inputs, dtype=np.float32))
    assert inputs.shape == (N_ROWS, 9), inputs.shape
    trace = bool(int(os.environ.get("BASS_KERNEL_TRACE", "0")))
    return _run(inputs, trace=trace)
